# revision 1
# baseline (speedup 1.0000x reference)
"""Quantized int8 conv2d (brevitas-style) on 8 TRN2 NeuronCores.

Data-parallel over batch (1 image / core). Per-tensor symmetric int8
quantization: local abs-max -> AllReduce(max) -> quantize -> 3x3 conv
(stride 1, pad 1) as PE col-tiled matmuls -> dequant + bias.

Key tricks:
- x is cached in SBUF as fp16 during the abs-max pass (single DRAM read).
- round(v) is computed as fp16(v + 1536) (fp16 RNE at the [1024,2048)
  binade has ulp=1 -> exact round-half-even, matching jnp.round). The
  +1536 offset rides through the conv matmuls and is cancelled exactly
  by a correction matmul per output row group using {0,1536} column
  patterns that replicate the zero-padding tap structure.
- Conv: output rows grouped by 4 (c = h%4). Each c is one PE column
  tile (0, 32c), all four concurrent, each accumulating into its own
  PSUM bank: 3 K=128 matmuls (kw taps; lhsT has zero row-blocks where
  the input row class is invalid) + 1 correction + (c=0/c=3) 3 windowed
  K=32 boundary matmuls against a staged copy of the adjacent block's
  edge row.
"""

import sys

if "/opt/trn_rl_repo" not in sys.path:
    sys.path.insert(0, "/opt/trn_rl_repo")

import numpy as np

import concourse.bass as bass
import concourse.bacc as bacc
import concourse.mybir as mybir
from concourse import tile
from concourse.bass_utils import run_bass_kernel_spmd

N_CORES = 8
C = 32
O = 32
H = 512
W = 512
F32 = mybir.dt.float32
F16 = mybir.dt.float16

MAXV = 127.0
RND = 1536.0

# per-kw output/rhs column windows: (out_start, rhs_start, n)
KW_COLS = {0: (1, 0, 511), 1: (0, 0, 512), 2: (0, 1, 511)}
G = 4  # q-blocks per DMA group


def build_nc(h=H):
    nc = bacc.Bacc(None, target_bir_lowering=False, debug=False)
    NQ = h // 4
    NG = NQ // G

    x_ext = nc.declare_dram_parameter("x", [C, h, W], F32, isOutput=False)
    w_ext = nc.declare_dram_parameter("weight", [O, C, 3, 3], F32, isOutput=False)
    b_ext = nc.declare_dram_parameter("bias", [O], F32, isOutput=False)
    out_ext = nc.declare_dram_parameter("out", [O, h, W], F32, isOutput=True)

    cc_in = nc.dram_tensor("cc_in", [128], F32)
    cc_out = nc.dram_tensor("cc_out", [128], F32, addr_space="Shared")

    with tile.TileContext(nc) as tc:
        with (
            tc.tile_pool(name="persist", bufs=1) as persist,
            tc.tile_pool(name="stage", bufs=3) as stage,
            tc.tile_pool(name="qx", bufs=6) as qxp,
            tc.tile_pool(name="outp", bufs=3) as outp,
            tc.tile_pool(name="spp", bufs=3) as spp,
            tc.tile_pool(name="snp", bufs=3) as snp,
            tc.tile_pool(name="ps0", bufs=2, space="PSUM") as psp0,
            tc.tile_pool(name="ps1", bufs=2, space="PSUM") as psp1,
            tc.tile_pool(name="ps2", bufs=2, space="PSUM") as psp2,
            tc.tile_pool(name="ps3", bufs=2, space="PSUM") as psp3,
        ):
            psps = [psp0, psp1, psp2, psp3]
            # ---------------- persistent SBUF tensors ----------------
            x4 = persist.tile([128, NQ * W], F16)
            maxes = persist.tile([128, NQ], F32)
            wsb = persist.tile([128, 288], F32)
            qw = persist.tile([128, 288], F16)
            tq = persist.tile([128, 288], F16)
            cw = persist.tile([128, 288], F16)
            w4 = persist.tile([128, 3 * 128], F16)   # main lhsT: kw blocks of (c,o)
            cwM = persist.tile([96, 3 * 128], F16)   # corr lhsT variants of (c,o)
            cb4 = persist.tile([96, W], F16)         # corr rhs patterns
            ones_l = persist.tile([1, 128], F32)
            bias_sb = persist.tile([128, 1], F32)
            gmax = persist.tile([128, 1], F32)
            gmax2 = persist.tile([128, 1], F32)
            gmaxr = persist.tile([1, 128], F32)
            wred = persist.tile([128, 1], F32)
            wredr = persist.tile([1, 128], F32)
            sg = persist.tile([1, 1], F32)
            sw = persist.tile([1, 1], F32)
            inv = persist.tile([1, 1], F32)
            invw = persist.tile([1, 1], F32)
            cwi = persist.tile([1, 1], F32)
            cqi = persist.tile([1, 1], F32)
            dqi = persist.tile([1, 1], F32)
            bc_in = persist.tile([1, 4], F32)
            bvec = persist.tile([128, 4], F32)
            cw_ap = persist.tile([128, 1], F32)
            s01 = persist.tile([128, 96], F16)
            s12 = persist.tile([128, 96], F16)
            sall = persist.tile([128, 96], F16)

            # ---------------- weight path (local) --------------------
            wv = w_ext[:, :, :, :].rearrange("o i kh kw -> i kh kw o")
            for g in range(4):
                nc.sync.dma_start(out=wsb[32 * g : 32 * g + 32, :], in_=wv)
            for cix in range(4):
                nc.sync.dma_start(
                    out=bias_sb[32 * cix : 32 * cix + 32, :], in_=b_ext[:, None]
                )
            nc.gpsimd.memset(ones_l[:, :], 1.0)
            nc.gpsimd.memset(w4[:, :], 0.0)
            nc.gpsimd.memset(cwM[:, :], 0.0)
            nc.gpsimd.memset(cb4[:, :], RND)
            nc.gpsimd.memset(cb4[0:32, 0:1], 0.0)
            nc.gpsimd.memset(cb4[64:96, W - 1 : W], 0.0)

            # sw = max |w| (X-reduce, fold partitions to a row, reduce again)
            nc.vector.tensor_reduce(
                out=wred[:, :], in_=wsb[:, :], axis=mybir.AxisListType.X,
                op=mybir.AluOpType.max, apply_absolute_value=True,
            )
            nc.sync.dma_start(out=wredr[0:1, 0:128], in_=wred[:, 0:1])
            nc.vector.tensor_reduce(
                out=sw[:, :], in_=wredr[:, :], axis=mybir.AxisListType.X,
                op=mybir.AluOpType.max,
            )
            nc.vector.reciprocal(invw[:, :], sw[:, :])
            nc.vector.tensor_scalar_mul(cwi[:, :], invw[:, :], MAXV)

            if True:
                bps = psp0.tile([128, 4], F32, tag="pst0")
                nc.tensor.matmul(bps[:, 0:1], ones_l[:, :], cwi[:, :])
                nc.vector.tensor_copy(cw_ap[:, :], bps[:, 0:1])

                # qw = round(w * 127/sw) via fp16 +1536 trick
                nc.scalar.activation(
                    out=tq[:, :], in_=wsb[:, :],
                    func=mybir.ActivationFunctionType.Copy,
                    scale=cw_ap[:, 0:1], bias=RND,
                )
                with nc.allow_low_precision("int8 values exact in fp16"):
                    nc.vector.tensor_scalar_add(qw[:, :], tq[:, :], -RND)
                    nc.vector.tensor_add(s01[:, :], qw[:, 0:96], qw[:, 96:192])
                    nc.vector.tensor_add(s12[:, :], qw[:, 96:192], qw[:, 192:288])
                    nc.vector.tensor_add(sall[:, :], s01[:, :], qw[:, 192:288])
                    nc.vector.tensor_scalar_mul(cw[:, 0:96], sall[:, :], -1.0)
                    nc.vector.tensor_scalar_mul(cw[:, 96:192], s12[:, :], -1.0)
                    nc.vector.tensor_scalar_mul(cw[:, 192:288], s01[:, :], -1.0)
                    # main lhsT: w4[32*hm+i, kw*128+c*32+o] = qw[o,i,hm-c+1,kw]
                    for cix in range(4):
                        for kw in range(3):
                            for kh in range(3):
                                hm = cix + kh - 1
                                if not (0 <= hm <= 3):
                                    continue
                                nc.vector.tensor_copy(
                                    w4[32 * hm : 32 * hm + 32,
                                       kw * 128 + cix * 32 : kw * 128 + cix * 32 + 32],
                                    qw[0:32, kh * 96 + kw * 32 : kh * 96 + kw * 32 + 32],
                                )
                    # corr lhsT: cwM[32*kw+i, vv*128+c*32+o], vv=0 mid,1 q0,2 qlast
                    for vv in range(3):
                        for cix in range(4):
                            v = 1 if (vv == 1 and cix == 0) else (
                                2 if (vv == 2 and cix == 3) else 0)
                            for kw in range(3):
                                nc.vector.tensor_copy(
                                    cwM[32 * kw : 32 * kw + 32,
                                        vv * 128 + cix * 32 : vv * 128 + cix * 32 + 32],
                                    cw[0:32, v * 96 + kw * 32 : v * 96 + kw * 32 + 32],
                                )

                # ------------- pass 1: stream x, absmax + fp16 cache --
                for q in range(NQ):
                    stg = stage.tile([128, W], F32)
                    xv = x_ext[:, 4 * q : 4 * q + 4, :].rearrange("i hm w -> hm i w")
                    eng = (nc.sync, nc.scalar, nc.gpsimd)[q % 3]
                    eng.dma_start(out=stg[:, :], in_=xv)
                    nc.scalar.activation(
                        out=x4[:, q * W : (q + 1) * W], in_=stg[:, :],
                        func=mybir.ActivationFunctionType.Copy,
                    )
                    nc.vector.tensor_reduce(
                        out=maxes[:, q : q + 1], in_=stg[:, :],
                        axis=mybir.AxisListType.X,
                        op=mybir.AluOpType.max, apply_absolute_value=True,
                    )

                nc.vector.tensor_reduce(
                    out=gmax[:, :], in_=maxes[:, :], axis=mybir.AxisListType.X,
                    op=mybir.AluOpType.max,
                )

                # ------------- all-reduce(max) across 8 cores ---------
                nc.sync.dma_start(out=cc_in[:, None], in_=gmax[:, :])
                nc.gpsimd.collective_compute(
                    "AllReduce", mybir.AluOpType.max,
                    replica_groups=[list(range(N_CORES))],
                    ins=[cc_in[:].opt()], outs=[cc_out[:].opt()],
                )
                nc.sync.dma_start(out=gmax2[:, :], in_=cc_out[:, None])
                nc.sync.dma_start(out=gmaxr[0:1, 0:128], in_=gmax2[:, 0:1])
                nc.vector.tensor_reduce(
                    out=sg[:, :], in_=gmaxr[:, :], axis=mybir.AxisListType.X,
                    op=mybir.AluOpType.max,
                )

                nc.vector.reciprocal(inv[:, :], sg[:, :])
                nc.vector.tensor_scalar_mul(cqi[:, :], inv[:, :], MAXV)
                nc.vector.tensor_mul(dqi[:, :], sg[:, :], sw[:, :])
                nc.vector.tensor_scalar_mul(dqi[:, :], dqi[:, :], 1.0 / (MAXV * MAXV))
                nc.vector.tensor_copy(bc_in[:, 0:1], cqi[:, :])
                nc.vector.tensor_copy(bc_in[:, 1:2], dqi[:, :])
                bps2 = psp1.tile([128, 4], F32, tag="pst1")
                nc.tensor.matmul(bps2[:, 0:2], ones_l[:, :], bc_in[:, 0:2])
                nc.vector.tensor_copy(bvec[:, 0:2], bps2[:, 0:2])
            cq_ap = bvec[:, 0:1]
            dq_ap = bvec[:, 1:2]

            # ---------------- pass 2 ----------------------------------
            qx_tiles = {}

            def quantize_block(j):
                t = qxp.tile([128, W], F16)
                nc.scalar.activation(
                    out=t[:, :], in_=x4[:, j * W : (j + 1) * W],
                    func=mybir.ActivationFunctionType.Copy,
                    scale=cq_ap, bias=RND,
                )
                qx_tiles[j] = t

            quantize_block(0)
            quantize_block(1)

            ot4 = None
            for q in range(NQ):
                if q + 2 <= NQ - 1:
                    quantize_block(q + 2)

                sp32 = sn32 = None
                if q > 0:
                    sp32 = spp.tile([32, W], F16)
                    nc.gpsimd.dma_start(out=sp32[:, :], in_=qx_tiles[q - 1][96:128, :])
                if q < NQ - 1:
                    sn32 = snp.tile([32, W], F16)
                    nc.gpsimd.dma_start(out=sn32[:, :], in_=qx_tiles[q + 1][0:32, :])

                cur = qx_tiles[q]
                pst = psps[q % 4].tile([128, W], F32, tag=f"pst{q % 4}")
                vv = 1 if q == 0 else (2 if q == NQ - 1 else 0)
                mms = []
                for kw in (1, 0, 2):
                    oc0, rc0, nn = KW_COLS[kw]
                    mms.append(
                        (w4[0:128, kw * 128 : kw * 128 + 128],
                         cur[0:128, rc0 : rc0 + nn],
                         pst[0:128, oc0 : oc0 + nn], (0, 0))
                    )
                if sp32 is not None:
                    for kw in (1, 0, 2):
                        oc0, rc0, nn = KW_COLS[kw]
                        mms.append(
                            (qw[0:32, kw * 32 : kw * 32 + 32],  # kh=0 -> c=0
                             sp32[0:32, rc0 : rc0 + nn],
                             pst[0:32, oc0 : oc0 + nn], (0, 0))
                        )
                if sn32 is not None:
                    for kw in (1, 0, 2):
                        oc0, rc0, nn = KW_COLS[kw]
                        mms.append(
                            (qw[0:32, 192 + kw * 32 : 192 + kw * 32 + 32],  # kh=2 -> c=3
                             sn32[0:32, rc0 : rc0 + nn],
                             pst[96:128, oc0 : oc0 + nn], (0, 96))
                        )
                mms.append(
                    (cwM[0:96, vv * 128 : vv * 128 + 128], cb4[0:96, 0:W],
                     pst[0:128, 0:W], (0, 0))
                )
                for mi, (lhsT, rhs, outap, tpos) in enumerate(mms):
                    nc.tensor.matmul(
                        outap, lhsT, rhs,
                        start=(mi == 0), stop=(mi == len(mms) - 1),
                        tile_position=tpos,
                    )

                ot4 = outp.tile([128, W], F32)
                nc.vector.tensor_scalar(
                    out=ot4[:, :], in0=pst[:, :],
                    scalar1=dq_ap, scalar2=bias_sb[:, 0:1],
                    op0=mybir.AluOpType.mult, op1=mybir.AluOpType.add,
                )
                ov = out_ext[:, 4 * q : 4 * q + 4, :].rearrange("o hm w -> hm o w")
                nc.sync.dma_start(out=ov, in_=ot4[:, :])

    nc.finalize()
    return nc


_NC_CACHE = {}


def kernel(x, weight, bias):
    x = np.ascontiguousarray(x, dtype=np.float32)
    weight = np.ascontiguousarray(weight, dtype=np.float32)
    bias = np.ascontiguousarray(bias, dtype=np.float32)
    if "nc" not in _NC_CACHE:
        _NC_CACHE["nc"] = build_nc()
    nc = _NC_CACHE["nc"]
    in_maps = [
        {"x": x[i], "weight": weight, "bias": bias} for i in range(N_CORES)
    ]
    res = run_bass_kernel_spmd(nc, in_maps, core_ids=list(range(N_CORES)))
    outs = [res.results[i]["out"] for i in range(N_CORES)]
    return np.stack(outs, axis=0)


if __name__ == "__main__":
    build_nc(h=32)
    print("build ok")



# revision 10
# speedup vs baseline: 1.2281x; 1.2281x over previous
"""Quantized int8 conv2d (brevitas-style) on 8 TRN2 NeuronCores.

Data-parallel over batch (1 image / core). Per-tensor symmetric int8
quantization: local abs-max -> AllReduce(max) -> quantize -> 3x3 conv
(stride 1, pad 1) as PE matmuls -> dequant + bias.

v2 structure (vs v1 baseline):
- Pass 1: batched gpsimd SWDGE cast-DMAs (fp32 DRAM -> fp16 SBUF cache,
  8 row-blocks per call) + fp16 abs-max on vector. No scalar cast pass.
- Dequant scale d = sx*sw/127^2 is folded into the weights post-AllReduce;
  bias rides as an extra K-row (ones rhs) of the correction matmul, so the
  PSUM->SBUF epilogue is a plain vector copy.
- Boundary matmuls read the neighbor qx tiles' partition slices directly
  (PE tile_position (96,0)/(0,96)), no staging copies.
- Output DMA batched 4 row-blocks per SWDGE call.

round(v) is computed as fp16(v + 1536) (fp16 RNE at the [1024,2048)
binade has ulp=1 -> exact round-half-even). The +1536 offset on qx rides
through the conv matmuls and is cancelled by the correction matmul using
{0,1536} column patterns replicating the zero-padding tap structure.
"""

import sys

if "/opt/trn_rl_repo" not in sys.path:
    sys.path.insert(0, "/opt/trn_rl_repo")

import numpy as np

import concourse.bass as bass
import concourse.bacc as bacc
import concourse.mybir as mybir
from concourse import tile
from concourse.bass_utils import run_bass_kernel_spmd

N_CORES = 8
C = 32
O = 32
H = 512
W = 512
F32 = mybir.dt.float32
F16 = mybir.dt.float16

MAXV = 127.0
RND = 1536.0

# per-kw output/rhs column windows: (out_start, rhs_start, n)
KW_COLS = {0: (1, 0, 511), 1: (0, 0, 512), 2: (0, 1, 511)}
G_IN = 16  # q-blocks per input DMA tile (4 calls per tile, one per hm)
G_OUT = 8  # q-blocks per output DMA group (4 calls per group)


def build_nc(h=H):
    nc = bacc.Bacc(None, target_bir_lowering=False, debug=False)
    NQ = h // 4
    NG = NQ // G_IN

    x_ext = nc.declare_dram_parameter("x", [C, h, W], F32, isOutput=False)
    w_ext = nc.declare_dram_parameter("weight", [O, C, 3, 3], F32, isOutput=False)
    b_ext = nc.declare_dram_parameter("bias", [O], F32, isOutput=False)
    out_ext = nc.declare_dram_parameter("out", [O, h, W], F32, isOutput=True)

    cc_in = nc.dram_tensor("cc_in", [128], F32)
    cc_out = nc.dram_tensor("cc_out", [128], F32, addr_space="Shared")

    with tile.TileContext(nc) as tc:
        with (
            tc.tile_pool(name="persist", bufs=1) as persist,
            tc.tile_pool(name="xg", bufs=NG) as xgp,
            tc.tile_pool(name="qx", bufs=6) as qxp,
            tc.tile_pool(name="og", bufs=2) as ogp,
            tc.tile_pool(name="ps0", bufs=2, space="PSUM") as psp0,
            tc.tile_pool(name="ps1", bufs=2, space="PSUM") as psp1,
            tc.tile_pool(name="ps2", bufs=2, space="PSUM") as psp2,
            tc.tile_pool(name="ps3", bufs=2, space="PSUM") as psp3,
        ):
            psps = [psp0, psp1, psp2, psp3]
            # ---------------- persistent SBUF tensors ----------------
            maxes = persist.tile([128, NG], F32)
            wsb = persist.tile([128, 288], F32)
            qw = persist.tile([128, 288], F16)
            qws = persist.tile([128, 288], F16)   # qw * d (post-AR)
            tq = persist.tile([128, 288], F16)
            cw = persist.tile([128, 288], F16)
            w4 = persist.tile([128, 3 * 128], F16)   # main lhsT: kw blocks of (c,o)
            w4s = persist.tile([128, 3 * 128], F16)  # w4 * d (post-AR)
            cwM = persist.tile([97, 3 * 128], F16)   # corr lhsT + bias row 96
            cwMs = persist.tile([97, 3 * 128], F16)  # scaled corr (bias row unscaled)
            cb4 = persist.tile([97, W], F16)         # corr rhs patterns + ones row
            ones_l = persist.tile([1, 128], F32)
            bias_sb = persist.tile([1, 32], F32)
            gmax = persist.tile([128, 1], F32)
            gmax2 = persist.tile([128, 1], F32)
            gmaxr = persist.tile([1, 128], F32)
            wred = persist.tile([128, 1], F32)
            wredr = persist.tile([1, 128], F32)
            sg = persist.tile([1, 1], F32)
            sw = persist.tile([1, 1], F32)
            inv = persist.tile([1, 1], F32)
            invw = persist.tile([1, 1], F32)
            cwi = persist.tile([1, 1], F32)
            cqi = persist.tile([1, 1], F32)
            dqi = persist.tile([1, 1], F32)
            bc_in = persist.tile([1, 4], F32)
            bvec = persist.tile([128, 4], F32)
            cw_ap = persist.tile([128, 1], F32)
            s01 = persist.tile([128, 96], F16)
            s12 = persist.tile([128, 96], F16)
            sall = persist.tile([128, 96], F16)

            # ---------------- weight path (local, pre-AR) -------------
            wv = w_ext[:, :, :, :].rearrange("o i kh kw -> i kh kw o")
            for g in range(4):
                nc.sync.dma_start(out=wsb[32 * g : 32 * g + 32, :], in_=wv)
            nc.sync.dma_start(out=bias_sb[0:1, :], in_=b_ext[None, :])
            nc.gpsimd.memset(ones_l[:, :], 1.0)
            nc.gpsimd.memset(w4[:, :], 0.0)
            nc.gpsimd.memset(cwM[:, :], 0.0)
            nc.gpsimd.memset(cb4[:, :], RND)
            nc.gpsimd.memset(cb4[0:32, 0:1], 0.0)
            nc.gpsimd.memset(cb4[64:96, W - 1 : W], 0.0)
            nc.gpsimd.memset(cb4[96:97, :], 1.0)  # bias rhs row

            # sw = max |w| (X-reduce, fold partitions to a row, reduce again)
            nc.vector.tensor_reduce(
                out=wred[:, :], in_=wsb[:, :], axis=mybir.AxisListType.X,
                op=mybir.AluOpType.max, apply_absolute_value=True,
            )
            nc.sync.dma_start(out=wredr[0:1, 0:128], in_=wred[:, 0:1])
            nc.vector.tensor_reduce(
                out=sw[:, :], in_=wredr[:, :], axis=mybir.AxisListType.X,
                op=mybir.AluOpType.max,
            )
            nc.vector.reciprocal(invw[:, :], sw[:, :])
            nc.vector.tensor_scalar_mul(cwi[:, :], invw[:, :], MAXV)

            bps = psp0.tile([128, 4], F32, tag="pst0")
            nc.tensor.matmul(bps[:, 0:1], ones_l[:, :], cwi[:, :])
            nc.vector.tensor_copy(cw_ap[:, :], bps[:, 0:1])

            # qw = round(w * 127/sw) via fp16 +1536 trick
            nc.scalar.activation(
                out=tq[:, :], in_=wsb[:, :],
                func=mybir.ActivationFunctionType.Copy,
                scale=cw_ap[:, 0:1], bias=RND,
            )
            with nc.allow_low_precision("int8 values exact in fp16"):
                nc.vector.tensor_scalar_add(qw[:, :], tq[:, :], -RND)
                nc.vector.tensor_add(s01[:, :], qw[:, 0:96], qw[:, 96:192])
                nc.vector.tensor_add(s12[:, :], qw[:, 96:192], qw[:, 192:288])
                nc.vector.tensor_add(sall[:, :], s01[:, :], qw[:, 192:288])
                nc.vector.tensor_scalar_mul(cw[:, 0:96], sall[:, :], -1.0)
                nc.vector.tensor_scalar_mul(cw[:, 96:192], s12[:, :], -1.0)
                nc.vector.tensor_scalar_mul(cw[:, 192:288], s01[:, :], -1.0)
                # main lhsT: w4[32*hm+i, kw*128+c*32+o] = qw[o,i,hm-c+1,kw]
                for cix in range(4):
                    for kw in range(3):
                        for kh in range(3):
                            hm = cix + kh - 1
                            if not (0 <= hm <= 3):
                                continue
                            nc.vector.tensor_copy(
                                w4[32 * hm : 32 * hm + 32,
                                   kw * 128 + cix * 32 : kw * 128 + cix * 32 + 32],
                                qw[0:32, kh * 96 + kw * 32 : kh * 96 + kw * 32 + 32],
                            )
                # corr lhsT: cwM[32*kw+i, vv*128+c*32+o], vv=0 mid,1 q0,2 qlast
                for vv in range(3):
                    for cix in range(4):
                        v = 1 if (vv == 1 and cix == 0) else (
                            2 if (vv == 2 and cix == 3) else 0)
                        for kw in range(3):
                            nc.vector.tensor_copy(
                                cwM[32 * kw : 32 * kw + 32,
                                    vv * 128 + cix * 32 : vv * 128 + cix * 32 + 32],
                                cw[0:32, v * 96 + kw * 32 : v * 96 + kw * 32 + 32],
                            )
                    # bias row (fp32 -> fp16 copy), replicated per c group
                    for cix in range(4):
                        nc.vector.tensor_copy(
                            cwM[96:97, vv * 128 + cix * 32 : vv * 128 + cix * 32 + 32],
                            bias_sb[0:1, :],
                        )

            # ------------- pass 1: cast-DMA x into fp16 cache + absmax
            xg = []
            for g in range(NG):
                t = xgp.tile([128, G_IN * W], F16)
                xv = x_ext[:, 4 * G_IN * g : 4 * G_IN * (g + 1), :].rearrange(
                    "i (r hm) w -> hm i r w", hm=4
                )
                for hm in range(4):
                    nc.gpsimd.dma_start(
                        out=t[32 * hm : 32 * hm + 32, :],
                        in_=xv[hm : hm + 1].opt(),
                    )
                with nc.allow_low_precision("absmax on fp16 cache"):
                    nc.vector.tensor_reduce(
                        out=maxes[:, g : g + 1], in_=t[:, :],
                        axis=mybir.AxisListType.X,
                        op=mybir.AluOpType.max, apply_absolute_value=True,
                    )
                xg.append(t)

            nc.vector.tensor_reduce(
                out=gmax[:, :], in_=maxes[:, :], axis=mybir.AxisListType.X,
                op=mybir.AluOpType.max,
            )

            # ------------- all-reduce(max) across 8 cores ---------
            nc.sync.dma_start(out=cc_in[:, None], in_=gmax[:, :])
            nc.gpsimd.collective_compute(
                "AllReduce", mybir.AluOpType.max,
                replica_groups=[list(range(N_CORES))],
                ins=[cc_in[:].opt()], outs=[cc_out[:].opt()],
            )
            nc.sync.dma_start(out=gmax2[:, :], in_=cc_out[:, None])
            nc.sync.dma_start(out=gmaxr[0:1, 0:128], in_=gmax2[:, 0:1])
            nc.vector.tensor_reduce(
                out=sg[:, :], in_=gmaxr[:, :], axis=mybir.AxisListType.X,
                op=mybir.AluOpType.max,
            )

            nc.vector.reciprocal(inv[:, :], sg[:, :])
            nc.vector.tensor_scalar_mul(cqi[:, :], inv[:, :], MAXV)
            nc.vector.tensor_mul(dqi[:, :], sg[:, :], sw[:, :])
            nc.vector.tensor_scalar_mul(dqi[:, :], dqi[:, :], 1.0 / (MAXV * MAXV))
            nc.vector.tensor_copy(bc_in[:, 0:1], cqi[:, :])
            nc.vector.tensor_copy(bc_in[:, 1:2], dqi[:, :])
            bps2 = psp1.tile([128, 4], F32, tag="pst1")
            nc.tensor.matmul(bps2[:, 0:2], ones_l[:, :], bc_in[:, 0:2])
            nc.vector.tensor_copy(bvec[:, 0:2], bps2[:, 0:2])
            cq_ap = bvec[:, 0:1]
            dvec = bvec[:, 1:2]

            # post-AR: fold dequant scale d into all weight lhsT tiles
            with nc.allow_low_precision("scaled int weights in fp16"):
                nc.vector.tensor_scalar_mul(w4s[:, :], w4[:, :], dvec)
                nc.vector.tensor_scalar_mul(qws[:, :], qw[:, :], dvec)
                nc.vector.tensor_scalar_mul(cwMs[0:96, :], cwM[0:96, :], bvec[0:96, 1:2])
                nc.vector.tensor_copy(cwMs[96:97, :], cwM[96:97, :])

            # ---------------- pass 2 ----------------------------------
            qx_tiles = {}

            def xs(j):
                return xg[j // G_IN][:, (j % G_IN) * W : (j % G_IN + 1) * W]

            def quantize_block(j):
                t = qxp.tile([128, W], F16)
                nc.scalar.activation(
                    out=t[:, :], in_=xs(j),
                    func=mybir.ActivationFunctionType.Copy,
                    scale=cq_ap, bias=RND,
                )
                qx_tiles[j] = t

            quantize_block(0)
            quantize_block(1)

            cur_og = None
            for q in range(NQ):
                if q + 2 <= NQ - 1:
                    quantize_block(q + 2)

                cur = qx_tiles[q]
                pst = psps[q % 4].tile([128, W], F32, tag=f"pst{q % 4}")
                vv = 1 if q == 0 else (2 if q == NQ - 1 else 0)
                mms = []
                for kw in (1, 0, 2):
                    oc0, rc0, nn = KW_COLS[kw]
                    mms.append(
                        (w4s[0:128, kw * 128 : kw * 128 + 128],
                         cur[0:128, rc0 : rc0 + nn],
                         pst[0:128, oc0 : oc0 + nn], (0, 0))
                    )
                if q > 0:
                    prev = qx_tiles[q - 1]
                    for kw in (1, 0, 2):
                        oc0, rc0, nn = KW_COLS[kw]
                        mms.append(
                            (qws[96:128, kw * 32 : kw * 32 + 32],  # kh=0 weights
                             prev[96:128, rc0 : rc0 + nn],
                             pst[0:32, oc0 : oc0 + nn], (96, 0))
                        )
                if q < NQ - 1:
                    nxt = qx_tiles[q + 1]
                    for kw in (1, 0, 2):
                        oc0, rc0, nn = KW_COLS[kw]
                        mms.append(
                            (qws[0:32, 192 + kw * 32 : 192 + kw * 32 + 32],  # kh=2
                             nxt[0:32, rc0 : rc0 + nn],
                             pst[96:128, oc0 : oc0 + nn], (0, 96))
                        )
                mms.append(
                    (cwMs[0:97, vv * 128 : vv * 128 + 128], cb4[0:97, 0:W],
                     pst[0:128, 0:W], (0, 0))
                )
                for mi, (lhsT, rhs, outap, tpos) in enumerate(mms):
                    nc.tensor.matmul(
                        outap, lhsT, rhs,
                        start=(mi == 0), stop=(mi == len(mms) - 1),
                        tile_position=tpos,
                    )

                # epilogue: PSUM (already dequantized + biased) -> SBUF group
                jo = q % G_OUT
                if jo == 0:
                    cur_og = ogp.tile([128, G_OUT * W], F32)
                nc.vector.tensor_copy(cur_og[:, jo * W : (jo + 1) * W], pst[:, :])
                if jo == G_OUT - 1:
                    g0 = q - (G_OUT - 1)
                    ov = out_ext[:, 4 * g0 : 4 * g0 + 4 * G_OUT, :].rearrange(
                        "o (r hm) w -> hm o r w", hm=4
                    )
                    for hm in range(4):
                        nc.gpsimd.dma_start(
                            out=ov[hm : hm + 1].opt(),
                            in_=cur_og[32 * hm : 32 * hm + 32, :],
                        )

    nc.finalize()
    return nc


_NC_CACHE = {}


def kernel(x, weight, bias):
    x = np.ascontiguousarray(x, dtype=np.float32)
    weight = np.ascontiguousarray(weight, dtype=np.float32)
    bias = np.ascontiguousarray(bias, dtype=np.float32)
    if "nc" not in _NC_CACHE:
        _NC_CACHE["nc"] = build_nc()
    nc = _NC_CACHE["nc"]
    in_maps = [
        {"x": x[i], "weight": weight, "bias": bias} for i in range(N_CORES)
    ]
    res = run_bass_kernel_spmd(nc, in_maps, core_ids=list(range(N_CORES)))
    outs = [res.results[i]["out"] for i in range(N_CORES)]
    return np.stack(outs, axis=0)


if __name__ == "__main__":
    build_nc(h=64)
    print("build ok")


# revision 19
# speedup vs baseline: 1.3392x; 1.0905x over previous
"""Quantized int8 conv2d (brevitas-style) on 8 TRN2 NeuronCores.

Data-parallel over batch (1 image / core). Per-tensor symmetric int8
quantization: local abs-max -> AllReduce(max) -> quantize -> 3x3 conv
(stride 1, pad 1) as PE matmuls -> dequant + bias.

v2 structure (vs v1 baseline):
- Pass 1: batched gpsimd SWDGE cast-DMAs (fp32 DRAM -> fp16 SBUF cache,
  8 row-blocks per call) + fp16 abs-max on vector. No scalar cast pass.
- Dequant scale d = sx*sw/127^2 is folded into the weights post-AllReduce;
  bias rides as an extra K-row (ones rhs) of the correction matmul, so the
  PSUM->SBUF epilogue is a plain vector copy.
- Boundary matmuls read the neighbor qx tiles' partition slices directly
  (PE tile_position (96,0)/(0,96)), no staging copies.
- Output DMA batched 4 row-blocks per SWDGE call.

round(v) is computed as fp16(v + 1536) (fp16 RNE at the [1024,2048)
binade has ulp=1 -> exact round-half-even). The +1536 offset on qx rides
through the conv matmuls and is cancelled by the correction matmul using
{0,1536} column patterns replicating the zero-padding tap structure.
"""

import sys

if "/opt/trn_rl_repo" not in sys.path:
    sys.path.insert(0, "/opt/trn_rl_repo")

import numpy as np

import concourse.bass as bass
import concourse.bacc as bacc
import concourse.mybir as mybir
from concourse import tile
from concourse.bass_utils import run_bass_kernel_spmd

N_CORES = 8
C = 32
O = 32
H = 512
W = 512
F32 = mybir.dt.float32
F16 = mybir.dt.float16

MAXV = 127.0
RND = 1536.0

# per-kw output/rhs column windows: (out_start, rhs_start, n)
KW_COLS = {0: (1, 0, 511), 1: (0, 0, 512), 2: (0, 1, 511)}
G_IN = 16  # q-blocks per input DMA tile (4 calls per tile, one per hm)
G_OUT = 8  # q-blocks per output DMA group (4 calls per group)


def build_nc(h=H):
    nc = bacc.Bacc(None, target_bir_lowering=False, debug=False)
    NQ = h // 4
    NG = NQ // G_IN

    x_ext = nc.declare_dram_parameter("x", [C, h, W], F32, isOutput=False)
    w_ext = nc.declare_dram_parameter("weight", [O, C, 3, 3], F32, isOutput=False)
    b_ext = nc.declare_dram_parameter("bias", [O], F32, isOutput=False)
    out_ext = nc.declare_dram_parameter("out", [O, h, W], F32, isOutput=True)

    cc_in = nc.dram_tensor("cc_in", [128], F32)
    cc_out = nc.dram_tensor("cc_out", [128], F32, addr_space="Shared")

    with tile.TileContext(nc) as tc:
        with (
            tc.tile_pool(name="persist", bufs=1) as persist,
            tc.tile_pool(name="xg", bufs=NG) as xgp,
            tc.tile_pool(name="qx", bufs=6) as qxp,
            tc.tile_pool(name="tr", bufs=3) as trp,
            tc.tile_pool(name="og", bufs=2) as ogp,
            tc.tile_pool(name="ps0", bufs=2, space="PSUM") as psp0,
            tc.tile_pool(name="ps1", bufs=2, space="PSUM") as psp1,
            tc.tile_pool(name="ps2", bufs=2, space="PSUM") as psp2,
            tc.tile_pool(name="ps3", bufs=2, space="PSUM") as psp3,
        ):
            psps = [psp0, psp1, psp2, psp3]
            # ---------------- persistent SBUF tensors ----------------
            maxes = persist.tile([128, NG], F32)
            wsb = persist.tile([128, 288], F32)
            qw = persist.tile([128, 288], F16)
            qws = persist.tile([128, 288], F16)   # qw * d (post-AR)
            tq = persist.tile([128, 288], F16)
            w4 = persist.tile([128, 3 * 128], F16)   # main lhsT: kw blocks of (c,o)
            w4s = persist.tile([128, 3 * 128], F16)  # w4 * d (post-AR)
            ones_l = persist.tile([1, 128], F32)
            bias_sb = persist.tile([128, 1], F32)
            gmax = persist.tile([128, 1], F32)
            gmax2 = persist.tile([128, 1], F32)
            gmaxr = persist.tile([1, 128], F32)
            wred = persist.tile([128, 1], F32)
            wredr = persist.tile([1, 128], F32)
            sg = persist.tile([1, 1], F32)
            sw = persist.tile([1, 1], F32)
            inv = persist.tile([1, 1], F32)
            invw = persist.tile([1, 1], F32)
            cwi = persist.tile([1, 1], F32)
            cqi = persist.tile([1, 1], F32)
            dqi = persist.tile([1, 1], F32)
            bc_in = persist.tile([1, 4], F32)
            bvec = persist.tile([128, 4], F32)
            cw_ap = persist.tile([128, 1], F32)
            s01 = persist.tile([128, 96], F16)
            s12 = persist.tile([128, 96], F16)
            sall = persist.tile([128, 96], F16)

            # ---------------- weight path (local, pre-AR) -------------
            wv = w_ext[:, :, :, :].rearrange("o i kh kw -> i kh kw o")
            for g in range(4):
                nc.sync.dma_start(out=wsb[32 * g : 32 * g + 32, :], in_=wv)
            for cix in range(4):
                nc.sync.dma_start(
                    out=bias_sb[32 * cix : 32 * cix + 32, :], in_=b_ext[:, None]
                )
            nc.gpsimd.memset(ones_l[:, :], 1.0)
            nc.gpsimd.memset(w4[:, :], 0.0)

            # sw = max |w| (X-reduce, fold partitions to a row, reduce again)
            nc.vector.tensor_reduce(
                out=wred[:, :], in_=wsb[:, :], axis=mybir.AxisListType.X,
                op=mybir.AluOpType.max, apply_absolute_value=True,
            )
            nc.sync.dma_start(out=wredr[0:1, 0:128], in_=wred[:, 0:1])
            nc.vector.tensor_reduce(
                out=sw[:, :], in_=wredr[:, :], axis=mybir.AxisListType.X,
                op=mybir.AluOpType.max,
            )
            nc.vector.reciprocal(invw[:, :], sw[:, :])
            nc.vector.tensor_scalar_mul(cwi[:, :], invw[:, :], MAXV)

            bps = psp0.tile([128, 4], F32, tag="pst0")
            nc.tensor.matmul(bps[:, 0:1], ones_l[:, :], cwi[:, :])
            nc.vector.tensor_copy(cw_ap[:, :], bps[:, 0:1])

            # qw = round(w * 127/sw) via fp16 +1536 trick
            nc.scalar.activation(
                out=tq[:, :], in_=wsb[:, :],
                func=mybir.ActivationFunctionType.Copy,
                scale=cw_ap[:, 0:1], bias=RND,
            )
            with nc.allow_low_precision("int8 values exact in fp16"):
                nc.vector.tensor_scalar_add(qw[:, :], tq[:, :], -RND)
                # main lhsT: w4[32*hm+i, kw*128+c*32+o] = qw[o,i,hm-c+1,kw]
                for cix in range(4):
                    for kw in range(3):
                        for kh in range(3):
                            hm = cix + kh - 1
                            if not (0 <= hm <= 3):
                                continue
                            nc.vector.tensor_copy(
                                w4[32 * hm : 32 * hm + 32,
                                   kw * 128 + cix * 32 : kw * 128 + cix * 32 + 32],
                                qw[0:32, kh * 96 + kw * 32 : kh * 96 + kw * 32 + 32],
                            )

            # ------------- pass 1: cast-DMA x into fp16 cache + absmax
            xg = []
            for g in range(NG):
                t = xgp.tile([128, G_IN * W], F16)
                xv = x_ext[:, 4 * G_IN * g : 4 * G_IN * (g + 1), :].rearrange(
                    "i (r hm) w -> hm i r w", hm=4
                )
                for hm in range(4):
                    nc.gpsimd.dma_start(
                        out=t[32 * hm : 32 * hm + 32, :],
                        in_=xv[hm : hm + 1].opt(),
                    )
                with nc.allow_low_precision("absmax on fp16 cache"):
                    nc.vector.tensor_reduce(
                        out=maxes[:, g : g + 1], in_=t[:, :],
                        axis=mybir.AxisListType.X,
                        op=mybir.AluOpType.max, apply_absolute_value=True,
                    )
                xg.append(t)

            nc.vector.tensor_reduce(
                out=gmax[:, :], in_=maxes[:, :], axis=mybir.AxisListType.X,
                op=mybir.AluOpType.max,
            )

            # ------------- all-reduce(max) across 8 cores ---------
            nc.sync.dma_start(out=cc_in[:, None], in_=gmax[:, :])
            nc.gpsimd.collective_compute(
                "AllReduce", mybir.AluOpType.max,
                replica_groups=[list(range(N_CORES))],
                ins=[cc_in[:].opt()], outs=[cc_out[:].opt()],
            )
            nc.sync.dma_start(out=gmax2[:, :], in_=cc_out[:, None])
            nc.sync.dma_start(out=gmaxr[0:1, 0:128], in_=gmax2[:, 0:1])
            nc.vector.tensor_reduce(
                out=sg[:, :], in_=gmaxr[:, :], axis=mybir.AxisListType.X,
                op=mybir.AluOpType.max,
            )

            nc.vector.reciprocal(inv[:, :], sg[:, :])
            nc.vector.tensor_scalar_mul(cqi[:, :], inv[:, :], MAXV)
            nc.vector.tensor_mul(dqi[:, :], sg[:, :], sw[:, :])
            nc.vector.tensor_scalar_mul(dqi[:, :], dqi[:, :], 1.0 / (MAXV * MAXV))
            nc.vector.tensor_copy(bc_in[:, 0:1], cqi[:, :])
            nc.vector.tensor_copy(bc_in[:, 1:2], dqi[:, :])
            bps2 = psp1.tile([128, 4], F32, tag="pst1")
            nc.tensor.matmul(bps2[:, 0:2], ones_l[:, :], bc_in[:, 0:2])
            nc.vector.tensor_copy(bvec[:, 0:2], bps2[:, 0:2])
            cq_ap = bvec[:, 0:1]
            dvec = bvec[:, 1:2]

            # post-AR: fold dequant scale d into all weight lhsT tiles
            with nc.allow_low_precision("scaled int weights in fp16"):
                nc.vector.tensor_scalar_mul(w4s[:, :], w4[:, :], dvec)
                nc.vector.tensor_scalar_mul(qws[:, :], qw[:, :], dvec)

            # ---------------- pass 2 ----------------------------------
            qx_tiles = {}

            def xs(j):
                return xg[j // G_IN][:, (j % G_IN) * W : (j % G_IN + 1) * W]

            def quantize_block(j):
                # round(x*cq) = fp16(x*cq + 1536) - 1536, exact in fp16
                tr = trp.tile([128, W], F16)
                nc.scalar.activation(
                    out=tr[:, :], in_=xs(j),
                    func=mybir.ActivationFunctionType.Copy,
                    scale=cq_ap, bias=RND,
                )
                t = qxp.tile([128, W], F16)
                with nc.allow_low_precision("int8 values exact in fp16"):
                    nc.vector.tensor_scalar_add(t[:, :], tr[:, :], -RND)
                qx_tiles[j] = t

            quantize_block(0)
            quantize_block(1)

            cur_og = None
            for q in range(NQ):
                if q + 2 <= NQ - 1:
                    quantize_block(q + 2)

                cur = qx_tiles[q]
                pst = psps[q % 4].tile([128, W], F32, tag=f"pst{q % 4}")
                mms = []
                for kw in (1, 0, 2):
                    oc0, rc0, nn = KW_COLS[kw]
                    mms.append(
                        (w4s[0:128, kw * 128 : kw * 128 + 128],
                         cur[0:128, rc0 : rc0 + nn],
                         pst[0:128, oc0 : oc0 + nn], (0, 0))
                    )
                if q > 0:
                    prev = qx_tiles[q - 1]
                    for kw in (1, 0, 2):
                        oc0, rc0, nn = KW_COLS[kw]
                        mms.append(
                            (qws[96:128, kw * 32 : kw * 32 + 32],  # kh=0 weights
                             prev[96:128, rc0 : rc0 + nn],
                             pst[0:32, oc0 : oc0 + nn], (96, 0))
                        )
                if q < NQ - 1:
                    nxt = qx_tiles[q + 1]
                    for kw in (1, 0, 2):
                        oc0, rc0, nn = KW_COLS[kw]
                        mms.append(
                            (qws[0:32, 192 + kw * 32 : 192 + kw * 32 + 32],  # kh=2
                             nxt[0:32, rc0 : rc0 + nn],
                             pst[96:128, oc0 : oc0 + nn], (0, 96))
                        )
                for mi, (lhsT, rhs, outap, tpos) in enumerate(mms):
                    nc.tensor.matmul(
                        outap, lhsT, rhs,
                        start=(mi == 0), stop=(mi == len(mms) - 1),
                        tile_position=tpos,
                    )

                # epilogue: PSUM (already dequantized) + bias -> SBUF group
                jo = q % G_OUT
                if jo == 0:
                    cur_og = ogp.tile([128, G_OUT * W], F32)
                nc.vector.tensor_scalar_add(
                    cur_og[:, jo * W : (jo + 1) * W], pst[:, :], bias_sb[:, 0:1]
                )
                if jo == G_OUT - 1:
                    g0 = q - (G_OUT - 1)
                    ov = out_ext[:, 4 * g0 : 4 * g0 + 4 * G_OUT, :].rearrange(
                        "o (r hm) w -> hm o r w", hm=4
                    )
                    for hm in range(4):
                        nc.gpsimd.dma_start(
                            out=ov[hm : hm + 1].opt(),
                            in_=cur_og[32 * hm : 32 * hm + 32, :],
                        )

    nc.finalize()
    return nc


_NC_CACHE = {}


def kernel(x, weight, bias):
    x = np.ascontiguousarray(x, dtype=np.float32)
    weight = np.ascontiguousarray(weight, dtype=np.float32)
    bias = np.ascontiguousarray(bias, dtype=np.float32)
    if "nc" not in _NC_CACHE:
        _NC_CACHE["nc"] = build_nc()
    nc = _NC_CACHE["nc"]
    in_maps = [
        {"x": x[i], "weight": weight, "bias": bias} for i in range(N_CORES)
    ]
    res = run_bass_kernel_spmd(nc, in_maps, core_ids=list(range(N_CORES)))
    outs = [res.results[i]["out"] for i in range(N_CORES)]
    return np.stack(outs, axis=0)


if __name__ == "__main__":
    build_nc(h=64)
    print("build ok")
